# revision 2
# baseline (speedup 1.0000x reference)
"""Trainium2 Bass kernel for the 3-room building thermal model scan.

Math (per step t, per batch row):
    x_{t+1} = x_t * exp(G_t)
    G_c     = iv_c * (R_tc + (M x_t)_c) + S_tc
where
    iv_c  = (H/C_c) / x_c              (maintained multiplicatively:
                                        iv <- iv * exp(-G), refreshed
                                        periodically via reciprocal)
    R_tc  = ee_c * u0[t]               (computed on device from u0)
    M     = [[0,e12,0],[e12,0,e23],[0,e23,0]]   (symmetric!)
    S_tc  = (H/C_c)*(es_c*u1 + eh_c*u_{2+c} + ec_c*u_{5+c})
            - (H/C_c)*(ee_c + [e12, e12+e23, e23]_c)    (host precomputed)

Sharding: pure data parallel, batch split 8 ways across cores.
Within a core: 1024 rows = 128 partitions x 8 groups; channels in the
free dimension; the output chunk tile doubles as the x-state history.
"""

import os
import sys

for _p in ("/opt/trn_rl_repo", "/root/.axon_site/_ro/trn_rl_repo"):
    if os.path.isdir(_p) and _p not in sys.path:
        sys.path.insert(0, _p)
        break

import numpy as np

H = 60.0
C = np.array([10665991.0, 27000000.0, 7953253.0], dtype=np.float64)
B, T, NCORES = 8192, 1024, 8
BL = B // NCORES     # rows per core
NG = BL // 128       # batch groups per core
TS = T - 1           # scan steps
REFRESH = 64         # iv refresh cadence (steps)

_cache = {}


def _default_chunks(ts):
    out = []
    left = ts
    while left > 0:
        c = min(128, left)
        out.append(c)
        left -= c
    return out


def _build(ts=TS, chunks=None):
    """Build + compile the Bass program for a `ts`-step scan."""
    import concourse.bacc as bacc
    import concourse.bass as bass
    import concourse.mybir as mybir
    from concourse.tile import TileContext

    if chunks is None:
        chunks = _default_chunks(ts)
    assert sum(chunks) == ts

    f32 = mybir.dt.float32
    mult = mybir.AluOpType.mult
    add = mybir.AluOpType.add
    subtract = mybir.AluOpType.subtract

    nc = bacc.Bacc("TRN2", target_bir_lowering=False, debug=False,
                   num_devices=NCORES)

    S_d = nc.dram_tensor("s_in", [BL, ts * 3], f32, kind="ExternalInput")
    U0_d = nc.dram_tensor("u0_in", [BL, ts], f32, kind="ExternalInput")
    X0_d = nc.dram_tensor("x0_in", [128, NG * 3], f32, kind="ExternalInput")
    IV0_d = nc.dram_tensor("iv_in", [128, NG * 3], f32, kind="ExternalInput")
    CN_d = nc.dram_tensor("cn_in", [128, 8], f32, kind="ExternalInput")
    O_d = nc.dram_tensor("o_out", [BL, ts * 3], f32, kind="ExternalOutput")

    Svw = S_d.rearrange("(g p) t -> p g t", p=128)
    U0vw = U0_d.rearrange("(g p) t -> p g t", p=128)
    Ovw = O_d.rearrange("(g p) t -> p g t", p=128)

    def view(tile_ap, off, dims):
        """Custom free-dim view of a [128, N] tile AP."""
        return bass.AP(tile_ap.tensor, tile_ap.offset + off,
                       [list(tile_ap.ap[0])] + [list(d) for d in dims])

    with TileContext(nc) as tc:
        with tc.tile_pool(name="const", bufs=1) as cpool, \
             tc.tile_pool(name="state", bufs=1) as spool, \
             tc.tile_pool(name="io", bufs=3) as iopool, \
             tc.tile_pool(name="work", bufs=2) as wpool, \
             tc.tile_pool(name="step", bufs=4) as pstep:

            CN = cpool.tile([128, 8], f32, tag="CN", name="CN")
            nc.sync.dma_start(CN, CN_d[:, :])
            X0 = cpool.tile([128, NG * 3], f32, tag="X0", name="X0")
            nc.sync.dma_start(X0, X0_d[:, :])
            IV = spool.tile([128, NG * 3], f32, tag="IV", name="IV")
            nc.sync.dma_start(IV, IV0_d[:, :])

            e12 = CN[:, 3:4]
            e23 = CN[:, 4:5]
            Kee = view(CN, 0, [[0, NG], [0, 1], [1, 3]])      # bcast [128,NG,1,3]
            Kh = view(CN, 5, [[0, NG], [1, 3]])               # bcast [128,NG,3]

            # per-step helper views ------------------------------------
            t0 = 0
            prev = (X0, 0, 3)  # (tile_ap, free offset of x, group stride)
            for k, TC in enumerate(chunks):
                Sc = iopool.tile([128, NG * TC * 3], f32, tag="Sc", name=f"Sc{k}")
                U0c = iopool.tile([128, NG * TC], f32, tag="U0c", name=f"U0c{k}")
                nc.sync.dma_start(
                    view(Sc, 0, [[TC * 3, NG], [1, TC * 3]]),
                    Svw[:, :, t0 * 3:(t0 + TC) * 3])
                nc.sync.dma_start(
                    view(U0c, 0, [[TC, NG], [1, TC]]),
                    U0vw[:, :, t0:t0 + TC])

                Rc = wpool.tile([128, NG * TC * 3], f32, tag="Rc", name=f"Rc{k}")
                # R = ee * u0   (broadcast channel dim from stride-0 read)
                nc.vector.tensor_tensor(
                    out=view(Rc, 0, [[TC * 3, NG], [3, TC], [1, 3]]),
                    in0=view(U0c, 0, [[TC, NG], [1, TC], [0, 3]]),
                    in1=view(CN, 0, [[0, NG], [0, TC], [1, 3]]),
                    op=mult)

                Oc = iopool.tile([128, NG * TC * 3], f32, tag="Oc", name=f"Oc{k}")

                for j in range(TC):
                    t = t0 + j
                    if j > 0:
                        xt, xo, gs = Oc, (j - 1) * 3, TC * 3
                    else:
                        xt, xo, gs = prev

                    x_full = view(xt, xo, [[gs, NG], [1, 3]])
                    x_rev1 = view(xt, xo + 1, [[gs, NG], [-1, 2]])
                    x_rev2 = view(xt, xo + 2, [[gs, NG], [-1, 2]])

                    W01 = view(Rc, j * 3, [[TC * 3, NG], [1, 2]])
                    W12 = view(Rc, j * 3 + 1, [[TC * 3, NG], [1, 2]])
                    Wf = view(Rc, j * 3, [[TC * 3, NG], [1, 3]])
                    St = view(Sc, j * 3, [[TC * 3, NG], [1, 3]])

                    EX = pstep.tile([128, 48], f32, tag="EX", name=f"EX{t}")
                    EE = pstep.tile([128, 48], f32, tag="EE", name=f"EE{t}")
                    Gp = view(EX, 0, [[3, NG], [1, 3]])       # left half
                    Gm = view(EX, 24, [[3, NG], [1, 3]])      # right half

                    # W = R + M x (two fused in-place ops)
                    nc.vector.scalar_tensor_tensor(
                        out=W01, in0=x_rev1, scalar=e12, in1=W01,
                        op0=mult, op1=add)
                    nc.vector.scalar_tensor_tensor(
                        out=W12, in0=x_rev2, scalar=e23, in1=W12,
                        op0=mult, op1=add)
                    # P = iv * W
                    nc.vector.tensor_tensor(out=Gp, in0=IV, in1=Wf, op=mult)
                    # Gm = -P - S ; Gp = P + S
                    nc.vector.scalar_tensor_tensor(
                        out=Gm, in0=Gp, scalar=-1.0, in1=St,
                        op0=mult, op1=subtract)
                    nc.vector.tensor_tensor(out=Gp, in0=Gp, in1=St, op=add)
                    # [E | 1/E] = exp([G | -G])
                    nc.scalar.activation(EE, EX,
                                         mybir.ActivationFunctionType.Exp)
                    # x_{t+1} = x * E  (written into the output chunk)
                    nc.vector.tensor_tensor(
                        out=view(Oc, j * 3, [[TC * 3, NG], [1, 3]]),
                        in0=x_full, in1=EE[:, 0:24], op=mult)
                    # iv update
                    if t % REFRESH == REFRESH - 1:
                        TMP = pstep.tile([128, NG * 3], f32, tag="TMP", name=f"TMP{t}")
                        nc.vector.reciprocal(
                            out=TMP,
                            in_=view(Oc, j * 3, [[TC * 3, NG], [1, 3]]))
                        nc.vector.tensor_tensor(
                            out=view(IV, 0, [[3, NG], [1, 3]]),
                            in0=view(TMP, 0, [[3, NG], [1, 3]]),
                            in1=Kh, op=mult)
                    else:
                        nc.gpsimd.tensor_tensor(
                            out=IV, in0=IV, in1=EE[:, 24:48], op=mult)

                nc.sync.dma_start(
                    Ovw[:, :, t0 * 3:(t0 + TC) * 3],
                    view(Oc, 0, [[TC * 3, NG], [1, TC * 3]]))
                prev = (Oc, (TC - 1) * 3, TC * 3)
                t0 += TC

    nc.compile()
    return nc


def _host_prep(x0, u, lam, ts=TS):
    """Host-side constant folding + per-(b,t) precompute + sharding."""
    lam64 = lam.astype(np.float64)
    e = np.exp(lam64)
    e12, e23 = e[0], e[1]
    ee, es, eh, ec = e[2:5], e[5:8], e[8:11], e[11:14]
    h = H / C  # [3] float64

    ces = (h * es).astype(np.float32)
    ceh = (h * eh).astype(np.float32)
    cec = (h * ec).astype(np.float32)
    bias = (-h * (ee + np.array([e12, e12 + e23, e23]))).astype(np.float32)

    uu = u[:, :ts, :]
    S = uu[:, :, 2:5] * ceh + uu[:, :, 5:8] * cec + uu[:, :, 1:2] * ces + bias
    S = np.ascontiguousarray(S.astype(np.float32)).reshape(B, ts * 3)
    u0 = np.ascontiguousarray(uu[:, :, 0].astype(np.float32))

    iv0 = (h[None, :] / x0.astype(np.float64)).astype(np.float32)

    cn_row = np.zeros(8, dtype=np.float32)
    cn_row[0:3] = ee.astype(np.float32)
    cn_row[3] = np.float32(e12)
    cn_row[4] = np.float32(e23)
    cn_row[5:8] = h.astype(np.float32)
    cn = np.tile(cn_row[None, :], (128, 1))

    def part_layout(a):  # [BL,3] -> [128, NG*3] with b = g*128+p
        return np.ascontiguousarray(
            a.reshape(NG, 128, 3).transpose(1, 0, 2).reshape(128, NG * 3))

    in_maps = []
    for c in range(NCORES):
        rows = slice(c * BL, (c + 1) * BL)
        in_maps.append({
            "s_in": S[rows],
            "u0_in": u0[rows],
            "x0_in": part_layout(x0[rows].astype(np.float32)),
            "iv_in": part_layout(iv0[rows]),
            "cn_in": cn,
        })
    return in_maps


def kernel(x0, u, lam, _ts=TS, _trace=False):
    from concourse.bass_utils import run_bass_kernel_spmd

    key = ("nc", _ts)
    if key not in _cache:
        _cache[key] = _build(_ts)
    nc = _cache[key]

    in_maps = _host_prep(x0, u, lam, ts=_ts)
    res = run_bass_kernel_spmd(nc, in_maps, core_ids=list(range(NCORES)),
                               trace=_trace)

    out = np.empty((B, T, 3), dtype=np.float32)
    out[:, 0, :] = x0
    out[:, 1:_ts + 1, :] = np.concatenate(
        [r["o_out"].reshape(BL, _ts, 3) for r in res.results], axis=0)
    if _ts < TS:
        out[:, _ts + 1:, :] = 0.0

    m = u[:, 1:, 0] < 1e-6
    if m.any():
        out[:, 1:, :][m] = -1.0

    if _trace:
        _cache["last_res"] = res
    return out


# revision 6
# speedup vs baseline: 1.7294x; 1.7294x over previous
"""Trainium2 Bass kernel for the 3-room building thermal model scan.

Math (per step t, per batch row):
    x_{t+1} = x_t * exp(G_t)
    G_c     = iv_c * (R_tc + (M x_t)_c) + S_tc
where
    iv_c  = (H/C_c) / x_c              (maintained multiplicatively:
                                        iv <- iv * exp(-G), refreshed
                                        periodically via reciprocal)
    R_tc  = ee_c * u0[t]               (computed on device from u0)
    M     = [[0,e12,0],[e12,0,e23],[0,e23,0]]   (symmetric!)
    S_tc  = (H/C_c)*(es_c*u1 + eh_c*u_{2+c} + ec_c*u_{5+c})
            - (H/C_c)*(ee_c + [e12, e12+e23, e23]_c)    (host precomputed)

Sharding: pure data parallel, batch split 8 ways across cores.
Within a core: 1024 rows = 128 partitions x 8 groups; channels in the
free dimension; the output chunk tile doubles as the x-state history.
"""

import os
import sys

for _p in ("/opt/trn_rl_repo", "/root/.axon_site/_ro/trn_rl_repo"):
    if os.path.isdir(_p) and _p not in sys.path:
        sys.path.insert(0, _p)
        break

import numpy as np

H = 60.0
C = np.array([10665991.0, 27000000.0, 7953253.0], dtype=np.float64)
B, T, NCORES = 8192, 1024, 8
BL = B // NCORES     # rows per core
NG = BL // 128       # batch groups per core
TS = T - 1           # scan steps
REFRESH = 64         # iv refresh cadence (steps)

_cache = {}


def _default_chunks(ts):
    out = []
    left = ts
    while left > 0:
        c = min(128, left)
        out.append(c)
        left -= c
    return out


def _register_custom_ops():
    """Register the two fused Horner-exp custom DVE ops.

    EXP3SQ_APPLY_ANT:  out = in0 * p(in1)^2,  p(g) = ((g/6 + 1/2)g + 1)g + 1
    EXPM3SQ_IV_ANT:    out = in0 * q(in1)^2,  q(g) = ((-g/6 + 1/2)g - 1)g + 1
    With g = G/2 these give x*exp(G) and iv*exp(-G) to ~G^4/192 rel error.
    """
    from concourse import dve_ops
    from concourse.dve_spec import C0, C1, C2, One, Spec, Src0, Src1, lower, sq
    from concourse.dve_table_gen import dve_ver_for
    from concourse.dve_uop import DveOpSpec

    made = {}
    for name, c2term, csign in (
        ("EXP3SQ_APPLY_ANT", One, 1.0),
        ("EXPM3SQ_IV_ANT", C2, -1.0),
    ):
        existing = [o for o in dve_ops.OPS if o.name == name]
        if existing:
            made[name] = existing[0]
            continue
        h = ((Src1 * C0 + C1) * Src1 + c2term) * Src1 + One
        body = sq(h) * Src0

        def _ref(in0, in1, s0, s1, imm2, _cs=csign):
            g = in1.astype(np.float32)
            inner = np.float32(imm2) if _cs < 0 else np.float32(1.0)
            p = ((g * np.float32(s0) + np.float32(s1)) * g + inner) * g \
                + np.float32(1.0)
            return (p * p).reshape(in0.shape) * in0.astype(np.float32)

        spec = Spec(body=body, reference=_ref)
        row = max(dve_ops._SUB_OPCODE_FOR_NAME.values()) + 1
        assert row < 0x20
        dve_ops._SUB_OPCODE_FOR_NAME[name] = row
        ver = dve_ver_for("TRN2")
        tmp = DveOpSpec(name=name, opcode=row, uops=lower(spec, ver=ver),
                        rd1_en=True)
        op = dve_ops.DveOp(name, spec, subdim=False,
                           uops_sha={ver: tmp.sha(ver)})
        dve_ops.OPS.append(op)
        dve_ops.CUSTOM_DVE_SPECS[name] = spec
        made[name] = op
    return made["EXP3SQ_APPLY_ANT"], made["EXPM3SQ_IV_ANT"]


def _build(ts=TS, chunks=None, lamvals=None):
    """Build + compile the Bass program for a `ts`-step scan."""
    import concourse.bacc as bacc
    import concourse.bass as bass
    import concourse.mybir as mybir
    from concourse.tile import TileContext

    e12i, e23i = lamvals
    EXP3SQ, EXPM3SQ = _register_custom_ops()

    if chunks is None:
        chunks = _default_chunks(ts)
    assert sum(chunks) == ts

    f32 = mybir.dt.float32
    mult = mybir.AluOpType.mult
    add = mybir.AluOpType.add
    subtract = mybir.AluOpType.subtract

    nc = bacc.Bacc("TRN2", target_bir_lowering=False, debug=False,
                   num_devices=NCORES)

    S_d = nc.dram_tensor("s_in", [BL, ts * 3], f32, kind="ExternalInput")
    U0_d = nc.dram_tensor("u0_in", [BL, ts], f32, kind="ExternalInput")
    X0_d = nc.dram_tensor("x0_in", [128, NG * 3], f32, kind="ExternalInput")
    IV0_d = nc.dram_tensor("iv_in", [128, NG * 3], f32, kind="ExternalInput")
    CN_d = nc.dram_tensor("cn_in", [128, 8], f32, kind="ExternalInput")
    O_d = nc.dram_tensor("o_out", [BL, ts * 3], f32, kind="ExternalOutput")

    Svw = S_d.rearrange("(g p) t -> p g t", p=128)
    U0vw = U0_d.rearrange("(g p) t -> p g t", p=128)
    Ovw = O_d.rearrange("(g p) t -> p g t", p=128)

    def view(tile_ap, off, dims):
        """Custom free-dim view of a [128, N] tile AP."""
        return bass.AP(tile_ap.tensor, tile_ap.offset + off,
                       [list(tile_ap.ap[0])] + [list(d) for d in dims])

    with TileContext(nc) as tc:
        with tc.tile_pool(name="const", bufs=1) as cpool, \
             tc.tile_pool(name="state", bufs=1) as spool, \
             tc.tile_pool(name="io", bufs=3) as iopool, \
             tc.tile_pool(name="work", bufs=2) as wpool, \
             tc.tile_pool(name="step", bufs=4) as pstep:

            CN = cpool.tile([128, 8], f32, tag="CN", name="CN")
            nc.sync.dma_start(CN, CN_d[:, :])
            X0 = cpool.tile([128, NG * 3], f32, tag="X0", name="X0")
            nc.sync.dma_start(X0, X0_d[:, :])
            IV = spool.tile([128, NG * 3], f32, tag="IV", name="IV")
            nc.sync.dma_start(IV, IV0_d[:, :])

            Kh = view(CN, 5, [[0, NG], [1, 3]])               # bcast [128,NG,3]

            # per-step helper views ------------------------------------
            t0 = 0
            prev = (X0, 0, 3)  # (tile_ap, free offset of x, group stride)
            for k, TC in enumerate(chunks):
                Sc = iopool.tile([128, NG * TC * 3], f32, tag="Sc", name=f"Sc{k}")
                U0c = iopool.tile([128, NG * TC], f32, tag="U0c", name=f"U0c{k}")
                nc.sync.dma_start(
                    view(Sc, 0, [[TC * 3, NG], [1, TC * 3]]),
                    Svw[:, :, t0 * 3:(t0 + TC) * 3])
                nc.sync.dma_start(
                    view(U0c, 0, [[TC, NG], [1, TC]]),
                    U0vw[:, :, t0:t0 + TC])

                Rc = wpool.tile([128, NG * TC * 3], f32, tag="Rc", name=f"Rc{k}")
                # R = ee * u0   (broadcast channel dim from stride-0 read)
                nc.gpsimd.tensor_tensor(
                    out=view(Rc, 0, [[TC * 3, NG], [3, TC], [1, 3]]),
                    in0=view(U0c, 0, [[TC, NG], [1, TC], [0, 3]]),
                    in1=view(CN, 0, [[0, NG], [0, TC], [1, 3]]),
                    op=mult)

                Oc = iopool.tile([128, NG * TC * 3], f32, tag="Oc", name=f"Oc{k}")

                for j in range(TC):
                    t = t0 + j
                    if j > 0:
                        xt, xo, gs = Oc, (j - 1) * 3, TC * 3
                    else:
                        xt, xo, gs = prev

                    x_full = view(xt, xo, [[gs, NG], [1, 3]])
                    x_rev1 = view(xt, xo + 1, [[gs, NG], [-1, 2]])
                    x_rev2 = view(xt, xo + 2, [[gs, NG], [-1, 2]])

                    W01 = view(Rc, j * 3, [[TC * 3, NG], [1, 2]])
                    W12 = view(Rc, j * 3 + 1, [[TC * 3, NG], [1, 2]])
                    Wf = view(Rc, j * 3, [[TC * 3, NG], [1, 3]])
                    St = view(Sc, j * 3, [[TC * 3, NG], [1, 3]])

                    EX = pstep.tile([128, 24], f32, tag="EX", name=f"EX{t}")
                    Gp = view(EX, 0, [[3, NG], [1, 3]])

                    # W = R + M x (two fused in-place ops)
                    nc.vector.scalar_tensor_tensor(
                        out=W01, in0=x_rev1, scalar=e12i, in1=W01,
                        op0=mult, op1=add)
                    nc.vector.scalar_tensor_tensor(
                        out=W12, in0=x_rev2, scalar=e23i, in1=W12,
                        op0=mult, op1=add)
                    # g = iv*W + S   (iv and S carry the 1/2 half-angle scale)
                    nc.vector.tensor_tensor(out=Gp, in0=IV, in1=Wf, op=mult)
                    nc.vector.tensor_tensor(out=Gp, in0=Gp, in1=St, op=add)
                    # x_{t+1} = x * p(g)^2  ~= x * exp(G)
                    nc.vector._custom_dve(
                        EXP3SQ,
                        out=view(Oc, j * 3, [[TC * 3, NG], [1, 3]]),
                        in0=x_full, in1=EX[:, :],
                        s0=1.0 / 6.0, s1=0.5)
                    # iv update
                    if t % REFRESH == REFRESH - 1:
                        TMP = pstep.tile([128, NG * 3], f32, tag="TMP", name=f"TMP{t}")
                        nc.vector.reciprocal(
                            out=TMP,
                            in_=view(Oc, j * 3, [[TC * 3, NG], [1, 3]]))
                        nc.vector.tensor_tensor(
                            out=view(IV, 0, [[3, NG], [1, 3]]),
                            in0=view(TMP, 0, [[3, NG], [1, 3]]),
                            in1=Kh, op=mult)
                    else:
                        # iv *= q(g)^2 ~= iv * exp(-G)
                        nc.vector._custom_dve(
                            EXPM3SQ, out=IV, in0=IV, in1=EX[:, :],
                            s0=-1.0 / 6.0, s1=0.5, imm2=-1.0)

                nc.sync.dma_start(
                    Ovw[:, :, t0 * 3:(t0 + TC) * 3],
                    view(Oc, 0, [[TC * 3, NG], [1, TC * 3]]))
                prev = (Oc, (TC - 1) * 3, TC * 3)
                t0 += TC

    nc.compile()
    return nc


def _host_prep(x0, u, lam, ts=TS):
    """Host-side constant folding + per-(b,t) precompute + sharding."""
    lam64 = lam.astype(np.float64)
    e = np.exp(lam64)
    e12, e23 = e[0], e[1]
    ee, es, eh, ec = e[2:5], e[5:8], e[8:11], e[11:14]
    h = H / C  # [3] float64

    ces = (h * es).astype(np.float32)
    ceh = (h * eh).astype(np.float32)
    cec = (h * ec).astype(np.float32)
    bias = (-h * (ee + np.array([e12, e12 + e23, e23]))).astype(np.float32)

    uu = u[:, :ts, :]
    S = uu[:, :, 2:5] * ceh + uu[:, :, 5:8] * cec + uu[:, :, 1:2] * ces + bias
    S = np.ascontiguousarray((0.5 * S).astype(np.float32)).reshape(B, ts * 3)
    u0 = np.ascontiguousarray(uu[:, :, 0].astype(np.float32))

    iv0 = (0.5 * h[None, :] / x0.astype(np.float64)).astype(np.float32)

    cn_row = np.zeros(8, dtype=np.float32)
    cn_row[0:3] = ee.astype(np.float32)
    cn_row[3] = np.float32(e12)
    cn_row[4] = np.float32(e23)
    cn_row[5:8] = (0.5 * h).astype(np.float32)
    cn = np.tile(cn_row[None, :], (128, 1))

    def part_layout(a):  # [BL,3] -> [128, NG*3] with b = g*128+p
        return np.ascontiguousarray(
            a.reshape(NG, 128, 3).transpose(1, 0, 2).reshape(128, NG * 3))

    in_maps = []
    for c in range(NCORES):
        rows = slice(c * BL, (c + 1) * BL)
        in_maps.append({
            "s_in": S[rows],
            "u0_in": u0[rows],
            "x0_in": part_layout(x0[rows].astype(np.float32)),
            "iv_in": part_layout(iv0[rows]),
            "cn_in": cn,
        })
    return in_maps


def kernel(x0, u, lam, _ts=TS, _trace=False):
    from concourse.bass_utils import run_bass_kernel_spmd

    e = np.exp(lam.astype(np.float64))
    lamvals = (float(np.float32(e[0])), float(np.float32(e[1])))
    key = ("nc", _ts, lamvals)
    if key not in _cache:
        _cache[key] = _build(_ts, lamvals=lamvals)
    nc = _cache[key]

    in_maps = _host_prep(x0, u, lam, ts=_ts)
    res = run_bass_kernel_spmd(nc, in_maps, core_ids=list(range(NCORES)),
                               trace=_trace)

    out = np.empty((B, T, 3), dtype=np.float32)
    out[:, 0, :] = x0
    out[:, 1:_ts + 1, :] = np.concatenate(
        [r["o_out"].reshape(BL, _ts, 3) for r in res.results], axis=0)
    if _ts < TS:
        out[:, _ts + 1:, :] = 0.0

    m = u[:, 1:, 0] < 1e-6
    if m.any():
        out[:, 1:, :][m] = -1.0

    if _trace:
        _cache["last_res"] = res
    return out
